# revision 11
# baseline (speedup 1.0000x reference)
"""MoE adapter layer (top-2 routing, bottleneck adapter experts) on 8 trn2 cores.

Strategy: data-parallel over batch rows. Gating (mean-pool -> top-2 -> softmax)
is computed on host (it is tiny); per selected expert the adapter weights are
packed/concatenated on host so each core runs a dense
  y = x + gelu(x @ wd_cat + bd_cat) @ wu_cat_scaled
over its 4 batch rows (4096 tokens). Matmuls run in float32r (full-rate fp32
on the PE, ~1e-4 precision), the residual path stays exact fp32.
importance/load replicate the reference gating math exactly on host.
"""

import numpy as np

# Problem shapes (hardcoded per task contract).
B, N, D, F, E, TOPK = 32, 1024, 768, 64, 8, 2
NCORES = 8
RPC = B // NCORES          # batch rows per core = 4
TOK = RPC * N              # tokens per core = 4096
TT = 512                   # tokens per tile
NT_PER_ROW = N // TT       # 2 tiles per batch row
DC = D // 128              # 6 contraction chunks
FK = TOPK * F              # 128 packed expert features

_STATE: dict = {}


def _build_program():
    import concourse.bacc as bacc
    import concourse.mybir as mybir
    import concourse.tile as tile
    from contextlib import ExitStack

    f32 = mybir.dt.float32
    f32r = mybir.dt.float32r

    nc = bacc.Bacc("TRN2", target_bir_lowering=False, debug=False)
    # All tensors live in transposed [d, t] layout; host pre/post-transposes.
    xt_d = nc.dram_tensor("xt", [D, TOK], f32r, kind="ExternalInput")
    wd_d = nc.dram_tensor("wd", [RPC, 128, DC * FK], f32r, kind="ExternalInput")
    wu_d = nc.dram_tensor("wu", [RPC, FK, D], f32r, kind="ExternalInput")
    bd_d = nc.dram_tensor("bd", [RPC, FK, 1], f32, kind="ExternalInput")
    yt_d = nc.dram_tensor("yt", [D, TOK], f32, kind="ExternalOutput")

    gelu = mybir.ActivationFunctionType.Gelu_apprx_tanh

    with ExitStack() as ctx:
        tc = ctx.enter_context(tile.TileContext(nc))
        wp = ctx.enter_context(tc.tile_pool(name="w", bufs=4))
        xtp = ctx.enter_context(tc.tile_pool(name="xt", bufs=4))
        hp = ctx.enter_context(tc.tile_pool(name="h", bufs=2))
        yp = ctx.enter_context(tc.tile_pool(name="y", bufs=3))
        ps_h = ctx.enter_context(tc.tile_pool(name="ps_h", bufs=2, space="PSUM"))
        ps_y = ctx.enter_context(tc.tile_pool(name="ps_y", bufs=6, space="PSUM"))

        for r in range(RPC):
            wd_sb = wp.tile([128, DC * FK], f32r, tag="wd")
            nc.sync.dma_start(wd_sb[:], wd_d[r])
            wu_sb = wp.tile([128, D], f32r, tag="wu")
            nc.sync.dma_start(wu_sb[:], wu_d[r])
            bd_sb = wp.tile([128, 1], f32, tag="bd")
            nc.sync.dma_start(bd_sb[:], bd_d[r])

            for n in range(NT_PER_ROW):
                t0 = r * N + n * TT
                # One DMA for the whole 512-token tile in [d, t] layout:
                # xT[(c p), t] -> sbuf [p, (c t)]
                xt_sb = xtp.tile([128, DC * TT], f32r, tag="xt")
                nc.sync.dma_start(
                    xt_sb[:].rearrange("p (c t) -> p c t", c=DC),
                    xt_d[:, t0: t0 + TT].rearrange("(c p) t -> p c t", p=128),
                )

                # Down projection: hT[fk, t] = sum_d wd_cat[d, fk] * xT[d, t]
                h_ps = ps_h.tile([128, TT], f32, tag="hps")
                for c in range(DC):
                    nc.tensor.matmul(
                        h_ps[:],
                        wd_sb[:, c * FK:(c + 1) * FK],
                        xt_sb[:, c * TT:(c + 1) * TT],
                        start=(c == 0),
                        stop=(c == DC - 1),
                    )
                h_sb = hp.tile([128, TT], f32r, tag="h")
                nc.scalar.activation(h_sb[:], h_ps[:], gelu, bias=bd_sb[:, 0:1])

                # Up projection + residual, per 128-wide d chunk (yT layout).
                yt_sb = yp.tile([128, DC * TT], f32, tag="y")
                for c in range(DC):
                    yt_ps = ps_y.tile([128, TT], f32, tag="yps")
                    nc.tensor.matmul(
                        yt_ps[:],
                        wu_sb[:, c * 128:(c + 1) * 128],
                        h_sb[:],
                        start=True, stop=True,
                    )
                    nc.vector.tensor_add(
                        yt_sb[:, c * TT:(c + 1) * TT], yt_ps[:],
                        xt_sb[:, c * TT:(c + 1) * TT].bitcast(f32),
                    )
                nc.sync.dma_start(
                    yt_d[:, t0: t0 + TT].rearrange("(c p) t -> p c t", p=128),
                    yt_sb[:].rearrange("p (c t) -> p c t", c=DC),
                )
    nc.finalize()
    return nc


def _make_runner(nc):
    """Cached jitted SPMD runner (mirrors bass2jax.run_bass_via_pjrt multi-core
    path, but reusable across calls so the NEFF compiles only once)."""
    import jax
    import concourse.mybir as mybir
    from jax.sharding import Mesh, PartitionSpec
    from jax.experimental.shard_map import shard_map
    from concourse import bass2jax

    bass2jax.install_neuronx_cc_hook()

    partition_name = (
        nc.partition_id_tensor.name if nc.partition_id_tensor else None
    )
    in_names: list = []
    out_names: list = []
    out_avals: list = []
    for alloc in nc.m.functions[0].allocations:
        if not isinstance(alloc, mybir.MemoryLocationSet):
            continue
        name = alloc.memorylocations[0].name
        if alloc.kind == "ExternalInput":
            if name != partition_name:
                in_names.append(name)
        elif alloc.kind == "ExternalOutput":
            out_names.append(name)
            out_avals.append(
                jax.core.ShapedArray(
                    tuple(alloc.tensor_shape), mybir.dt.np(alloc.dtype)
                )
            )
    n_params = len(in_names)
    n_outs = len(out_avals)
    all_in_names = tuple(
        in_names + out_names + ([partition_name] if partition_name else [])
    )

    def _body(*args):
        operands = list(args)
        if partition_name is not None:
            operands.append(bass2jax.partition_id_tensor())
        outs = bass2jax._bass_exec_p.bind(
            *operands,
            out_avals=tuple(out_avals),
            in_names=all_in_names,
            out_names=tuple(out_names),
            lowering_input_output_aliases=(),
            sim_require_finite=True,
            sim_require_nnan=True,
            nc=nc,
        )
        return tuple(outs)

    devices = jax.devices()[:NCORES]
    mesh = Mesh(np.asarray(devices), ("core",))
    in_specs = (PartitionSpec("core"),) * (n_params + n_outs)
    out_specs = (PartitionSpec("core"),) * n_outs
    donate = tuple(range(n_params, n_params + n_outs))
    sharded = jax.jit(
        shard_map(_body, mesh=mesh, in_specs=in_specs, out_specs=out_specs,
                  check_rep=False),
        donate_argnums=donate,
        keep_unused=True,
    )

    def run(in_maps):
        concat_in = [
            np.concatenate([np.asarray(m[name]) for m in in_maps], axis=0)
            for name in in_names
        ]
        concat_zeros = [
            np.zeros((NCORES * a.shape[0], *a.shape[1:]), a.dtype)
            for a in out_avals
        ]
        out_arrs = sharded(*concat_in, *concat_zeros)
        return [
            {
                name: np.asarray(out_arrs[i]).reshape(
                    NCORES, *out_avals[i].shape
                )[c]
                for i, name in enumerate(out_names)
            }
            for c in range(NCORES)
        ]

    return run


def _get_state():
    if "run" not in _STATE:
        nc = _build_program()
        _STATE["nc"] = nc
        _STATE["run"] = _make_runner(nc)
    return _STATE


def _gating(tokens, w_gate):
    """Replicates the reference gating math on host (float32 numpy)."""
    pooled = tokens.mean(axis=1, dtype=np.float32)          # [B, D]
    logits = pooled @ w_gate                                # [B, E]
    order = np.argsort(-logits, axis=1, kind="stable")
    topk_idx = order[:, :TOPK]                              # [B, K]
    topk_vals = np.take_along_axis(logits, topk_idx, axis=1)
    mx = topk_vals.max(axis=1, keepdims=True)
    ex = np.exp(topk_vals - mx)
    topk_gates = (ex / ex.sum(axis=1, keepdims=True)).astype(np.float32)
    gates = np.zeros((B, E), np.float32)
    gates[np.arange(B)[:, None], topk_idx] = topk_gates
    importance = gates.sum(axis=0)
    load = (gates > 0).sum(axis=0).astype(np.float32)
    return topk_idx, topk_gates, importance, load


def _pack_inputs(tokens, w_down, b_down, w_up, topk_idx, topk_gates):
    """Builds the 8 per-core input maps (weights gathered per batch row)."""
    sel_wd = w_down[topk_idx]                               # [B, K, D, F]
    wd_cat = np.transpose(sel_wd, (0, 2, 1, 3)).reshape(B, D, FK)
    # lhsT layout: [d_within_chunk, (chunk, fk)]
    wdh = np.ascontiguousarray(
        wd_cat.reshape(B, DC, 128, FK).transpose(0, 2, 1, 3).reshape(B, 128, DC * FK)
    )

    sel_wu = w_up[topk_idx]                                 # [B, K, F, D]
    wu_scaled = sel_wu * topk_gates[:, :, None, None]
    wuh = np.ascontiguousarray(wu_scaled.reshape(B, FK, D))

    bdh = b_down[topk_idx].reshape(B, FK, 1).astype(np.float32)

    in_maps = []
    for c in range(NCORES):
        rows = slice(c * RPC, (c + 1) * RPC)
        in_maps.append({
            "xt": np.ascontiguousarray(tokens[rows].reshape(TOK, D).T),
            "wd": np.ascontiguousarray(wdh[rows]),
            "wu": np.ascontiguousarray(wuh[rows]),
            "bd": np.ascontiguousarray(bdh[rows]),
        })
    return in_maps


def kernel(tokens, w_gate, w_down, b_down, w_up, b_up):
    tokens = np.asarray(tokens, dtype=np.float32)
    w_gate = np.asarray(w_gate, dtype=np.float32)
    w_down = np.asarray(w_down, dtype=np.float32)
    b_down = np.asarray(b_down, dtype=np.float32)
    w_up = np.asarray(w_up, dtype=np.float32)
    b_up = np.asarray(b_up, dtype=np.float32)

    topk_idx, topk_gates, importance, load = _gating(tokens, w_gate)
    in_maps = _pack_inputs(tokens, w_down, b_down, w_up, topk_idx, topk_gates)

    results = _get_state()["run"](in_maps)
    combined = np.concatenate(
        [results[c]["yt"].T for c in range(NCORES)], axis=0
    ).reshape(B, N, D)

    if np.any(b_up):
        brow = (topk_gates[..., None] * b_up[topk_idx]).sum(axis=1)  # [B, D]
        combined = combined + brow[:, None, :]

    return combined, importance, load


# revision 13
# speedup vs baseline: 1.0211x; 1.0211x over previous
"""MoE adapter layer (top-2 routing, bottleneck adapter experts) on 8 trn2 cores.

Strategy: data-parallel over batch rows. Gating (mean-pool -> top-2 -> softmax)
is computed on host (it is tiny); per selected expert the adapter weights are
packed/concatenated on host so each core runs a dense
  y = x + gelu(x @ wd_cat + bd_cat) @ wu_cat_scaled
over its 4 batch rows (4096 tokens). Matmuls run in float32r (full-rate fp32
on the PE, ~1e-4 precision), the residual path stays exact fp32.
importance/load replicate the reference gating math exactly on host.
"""

import numpy as np

# Problem shapes (hardcoded per task contract).
B, N, D, F, E, TOPK = 32, 1024, 768, 64, 8, 2
NCORES = 8
RPC = B // NCORES          # batch rows per core = 4
TOK = RPC * N              # tokens per core = 4096
TT = 512                   # tokens per tile
NT_PER_ROW = N // TT       # 2 tiles per batch row
DC = D // 128              # 6 contraction chunks
FK = TOPK * F              # 128 packed expert features

_STATE: dict = {}


def _build_program():
    import concourse.bacc as bacc
    import concourse.mybir as mybir
    import concourse.tile as tile
    from contextlib import ExitStack

    f32 = mybir.dt.float32
    f32r = mybir.dt.float32r

    nc = bacc.Bacc("TRN2", target_bir_lowering=False, debug=False)
    # All tensors live in transposed [d, t] layout; host pre/post-transposes.
    xt_d = nc.dram_tensor("xt", [D, TOK], f32r, kind="ExternalInput")
    wd_d = nc.dram_tensor("wd", [RPC, 128, DC * FK], f32r, kind="ExternalInput")
    wu_d = nc.dram_tensor("wu", [RPC, FK, D], f32r, kind="ExternalInput")
    bd_d = nc.dram_tensor("bd", [RPC, FK, 1], f32, kind="ExternalInput")
    yt_d = nc.dram_tensor("yt", [D, TOK], f32, kind="ExternalOutput")

    gelu = mybir.ActivationFunctionType.Gelu_apprx_tanh

    with ExitStack() as ctx:
        tc = ctx.enter_context(tile.TileContext(nc))
        wp = ctx.enter_context(tc.tile_pool(name="w", bufs=4))
        xtp = ctx.enter_context(tc.tile_pool(name="xt", bufs=4))
        hp = ctx.enter_context(tc.tile_pool(name="h", bufs=2))
        yp = ctx.enter_context(tc.tile_pool(name="y", bufs=3))
        ps_h = ctx.enter_context(tc.tile_pool(name="ps_h", bufs=2, space="PSUM"))
        ps_y = ctx.enter_context(tc.tile_pool(name="ps_y", bufs=6, space="PSUM"))

        for r in range(RPC):
            wd_sb = wp.tile([128, DC * FK], f32r, tag="wd")
            nc.sync.dma_start(wd_sb[:], wd_d[r])
            wu_sb = wp.tile([128, D], f32r, tag="wu")
            nc.sync.dma_start(wu_sb[:], wu_d[r])
            bd_sb = wp.tile([128, 1], f32, tag="bd")
            nc.sync.dma_start(bd_sb[:], bd_d[r])

            for n in range(NT_PER_ROW):
                t0 = r * N + n * TT
                # One DMA for the whole 512-token tile in [d, t] layout:
                # xT[(c p), t] -> sbuf [p, (c t)]
                xt_sb = xtp.tile([128, DC * TT], f32r, tag="xt")
                for g in range(2):
                    c0, c1 = g * (DC // 2), (g + 1) * (DC // 2)
                    nc.sync.dma_start(
                        xt_sb[:, c0 * TT: c1 * TT].rearrange(
                            "p (c t) -> p c t", c=DC // 2),
                        xt_d[c0 * 128: c1 * 128, t0: t0 + TT].rearrange(
                            "(c p) t -> p c t", p=128),
                    )

                # Down projection: hT[fk, t] = sum_d wd_cat[d, fk] * xT[d, t]
                h_ps = ps_h.tile([128, TT], f32, tag="hps")
                for c in range(DC):
                    nc.tensor.matmul(
                        h_ps[:],
                        wd_sb[:, c * FK:(c + 1) * FK],
                        xt_sb[:, c * TT:(c + 1) * TT],
                        start=(c == 0),
                        stop=(c == DC - 1),
                    )
                h_sb = hp.tile([128, TT], f32r, tag="h")
                nc.scalar.activation(h_sb[:], h_ps[:], gelu, bias=bd_sb[:, 0:1])

                # Up projection + residual, per 128-wide d chunk (yT layout).
                yt_sb = yp.tile([128, DC * TT], f32, tag="y")
                for c in range(DC):
                    yt_ps = ps_y.tile([128, TT], f32, tag="yps")
                    nc.tensor.matmul(
                        yt_ps[:],
                        wu_sb[:, c * 128:(c + 1) * 128],
                        h_sb[:],
                        start=True, stop=True,
                    )
                    nc.vector.tensor_add(
                        yt_sb[:, c * TT:(c + 1) * TT], yt_ps[:],
                        xt_sb[:, c * TT:(c + 1) * TT].bitcast(f32),
                    )
                for g in range(2):
                    c0, c1 = g * (DC // 2), (g + 1) * (DC // 2)
                    nc.sync.dma_start(
                        yt_d[c0 * 128: c1 * 128, t0: t0 + TT].rearrange(
                            "(c p) t -> p c t", p=128),
                        yt_sb[:, c0 * TT: c1 * TT].rearrange(
                            "p (c t) -> p c t", c=DC // 2),
                    )
    nc.finalize()
    return nc


def _make_runner(nc):
    """Cached jitted SPMD runner (mirrors bass2jax.run_bass_via_pjrt multi-core
    path, but reusable across calls so the NEFF compiles only once)."""
    import jax
    import concourse.mybir as mybir
    from jax.sharding import Mesh, PartitionSpec
    from jax.experimental.shard_map import shard_map
    from concourse import bass2jax

    bass2jax.install_neuronx_cc_hook()

    partition_name = (
        nc.partition_id_tensor.name if nc.partition_id_tensor else None
    )
    in_names: list = []
    out_names: list = []
    out_avals: list = []
    for alloc in nc.m.functions[0].allocations:
        if not isinstance(alloc, mybir.MemoryLocationSet):
            continue
        name = alloc.memorylocations[0].name
        if alloc.kind == "ExternalInput":
            if name != partition_name:
                in_names.append(name)
        elif alloc.kind == "ExternalOutput":
            out_names.append(name)
            out_avals.append(
                jax.core.ShapedArray(
                    tuple(alloc.tensor_shape), mybir.dt.np(alloc.dtype)
                )
            )
    n_params = len(in_names)
    n_outs = len(out_avals)
    all_in_names = tuple(
        in_names + out_names + ([partition_name] if partition_name else [])
    )

    def _body(*args):
        operands = list(args)
        if partition_name is not None:
            operands.append(bass2jax.partition_id_tensor())
        outs = bass2jax._bass_exec_p.bind(
            *operands,
            out_avals=tuple(out_avals),
            in_names=all_in_names,
            out_names=tuple(out_names),
            lowering_input_output_aliases=(),
            sim_require_finite=True,
            sim_require_nnan=True,
            nc=nc,
        )
        return tuple(outs)

    devices = jax.devices()[:NCORES]
    mesh = Mesh(np.asarray(devices), ("core",))
    in_specs = (PartitionSpec("core"),) * (n_params + n_outs)
    out_specs = (PartitionSpec("core"),) * n_outs
    donate = tuple(range(n_params, n_params + n_outs))
    sharded = jax.jit(
        shard_map(_body, mesh=mesh, in_specs=in_specs, out_specs=out_specs,
                  check_rep=False),
        donate_argnums=donate,
        keep_unused=True,
    )

    def run(in_maps):
        concat_in = [
            np.concatenate([np.asarray(m[name]) for m in in_maps], axis=0)
            for name in in_names
        ]
        concat_zeros = [
            np.zeros((NCORES * a.shape[0], *a.shape[1:]), a.dtype)
            for a in out_avals
        ]
        out_arrs = sharded(*concat_in, *concat_zeros)
        return [
            {
                name: np.asarray(out_arrs[i]).reshape(
                    NCORES, *out_avals[i].shape
                )[c]
                for i, name in enumerate(out_names)
            }
            for c in range(NCORES)
        ]

    return run


def _get_state():
    if "run" not in _STATE:
        nc = _build_program()
        _STATE["nc"] = nc
        _STATE["run"] = _make_runner(nc)
    return _STATE


def _gating(tokens, w_gate):
    """Replicates the reference gating math on host (float32 numpy)."""
    pooled = tokens.mean(axis=1, dtype=np.float32)          # [B, D]
    logits = pooled @ w_gate                                # [B, E]
    order = np.argsort(-logits, axis=1, kind="stable")
    topk_idx = order[:, :TOPK]                              # [B, K]
    topk_vals = np.take_along_axis(logits, topk_idx, axis=1)
    mx = topk_vals.max(axis=1, keepdims=True)
    ex = np.exp(topk_vals - mx)
    topk_gates = (ex / ex.sum(axis=1, keepdims=True)).astype(np.float32)
    gates = np.zeros((B, E), np.float32)
    gates[np.arange(B)[:, None], topk_idx] = topk_gates
    importance = gates.sum(axis=0)
    load = (gates > 0).sum(axis=0).astype(np.float32)
    return topk_idx, topk_gates, importance, load


def _pack_inputs(tokens, w_down, b_down, w_up, topk_idx, topk_gates):
    """Builds the 8 per-core input maps (weights gathered per batch row)."""
    sel_wd = w_down[topk_idx]                               # [B, K, D, F]
    wd_cat = np.transpose(sel_wd, (0, 2, 1, 3)).reshape(B, D, FK)
    # lhsT layout: [d_within_chunk, (chunk, fk)]
    wdh = np.ascontiguousarray(
        wd_cat.reshape(B, DC, 128, FK).transpose(0, 2, 1, 3).reshape(B, 128, DC * FK)
    )

    sel_wu = w_up[topk_idx]                                 # [B, K, F, D]
    wu_scaled = sel_wu * topk_gates[:, :, None, None]
    wuh = np.ascontiguousarray(wu_scaled.reshape(B, FK, D))

    bdh = b_down[topk_idx].reshape(B, FK, 1).astype(np.float32)

    in_maps = []
    for c in range(NCORES):
        rows = slice(c * RPC, (c + 1) * RPC)
        in_maps.append({
            "xt": np.ascontiguousarray(tokens[rows].reshape(TOK, D).T),
            "wd": np.ascontiguousarray(wdh[rows]),
            "wu": np.ascontiguousarray(wuh[rows]),
            "bd": np.ascontiguousarray(bdh[rows]),
        })
    return in_maps


def kernel(tokens, w_gate, w_down, b_down, w_up, b_up):
    tokens = np.asarray(tokens, dtype=np.float32)
    w_gate = np.asarray(w_gate, dtype=np.float32)
    w_down = np.asarray(w_down, dtype=np.float32)
    b_down = np.asarray(b_down, dtype=np.float32)
    w_up = np.asarray(w_up, dtype=np.float32)
    b_up = np.asarray(b_up, dtype=np.float32)

    topk_idx, topk_gates, importance, load = _gating(tokens, w_gate)
    in_maps = _pack_inputs(tokens, w_down, b_down, w_up, topk_idx, topk_gates)

    results = _get_state()["run"](in_maps)
    combined = np.concatenate(
        [results[c]["yt"].T for c in range(NCORES)], axis=0
    ).reshape(B, N, D)

    if np.any(b_up):
        brow = (topk_gates[..., None] * b_up[topk_idx]).sum(axis=1)  # [B, D]
        combined = combined + brow[:, None, :]

    return combined, importance, load


# revision 15
# speedup vs baseline: 1.0960x; 1.0734x over previous
"""MoE adapter layer (top-2 routing, bottleneck adapter experts) on 8 trn2 cores.

Strategy: data-parallel over batch rows. Gating (mean-pool -> top-2 -> softmax)
is computed on host (it is tiny); per selected expert the adapter weights are
packed/concatenated on host so each core runs a dense
  y = x + gelu(x @ wd_cat + bd_cat) @ wu_cat_scaled
over its 4 batch rows (4096 tokens). Matmuls run in float32r (full-rate fp32
on the PE, ~1e-4 precision), the residual path stays exact fp32.
importance/load replicate the reference gating math exactly on host.
"""

import numpy as np

# Problem shapes (hardcoded per task contract).
B, N, D, F, E, TOPK = 32, 1024, 768, 64, 8, 2
NCORES = 8
RPC = B // NCORES          # batch rows per core = 4
TOK = RPC * N              # tokens per core = 4096
TT = 512                   # tokens per tile
NT_PER_ROW = N // TT       # 2 tiles per batch row
DC = D // 128              # 6 contraction chunks
FK = TOPK * F              # 128 packed expert features

_STATE: dict = {}


def _build_program():
    import concourse.bacc as bacc
    import concourse.mybir as mybir
    import concourse.tile as tile
    from contextlib import ExitStack

    f32 = mybir.dt.float32
    f32r = mybir.dt.float32r

    nc = bacc.Bacc("TRN2", target_bir_lowering=False, debug=False)
    # All tensors live in transposed [d, t] layout; host pre/post-transposes.
    xt_d = nc.dram_tensor("xt", [D, TOK], f32r, kind="ExternalInput")
    wd_d = nc.dram_tensor("wd", [RPC, 128, DC * FK], f32r, kind="ExternalInput")
    wu_d = nc.dram_tensor("wu", [RPC, FK, D], f32r, kind="ExternalInput")
    bd_d = nc.dram_tensor("bd", [RPC, FK, 1], f32, kind="ExternalInput")
    yt_d = nc.dram_tensor("yt", [D, TOK], f32, kind="ExternalOutput")

    gelu = mybir.ActivationFunctionType.Gelu_apprx_tanh

    with ExitStack() as ctx:
        tc = ctx.enter_context(tile.TileContext(nc))
        wp = ctx.enter_context(tc.tile_pool(name="w", bufs=4))
        xtp = ctx.enter_context(tc.tile_pool(name="xt", bufs=6))
        hp = ctx.enter_context(tc.tile_pool(name="h", bufs=2))
        yp = ctx.enter_context(tc.tile_pool(name="y", bufs=4))
        ps_h = ctx.enter_context(tc.tile_pool(name="ps_h", bufs=2, space="PSUM"))
        ps_y = ctx.enter_context(tc.tile_pool(name="ps_y", bufs=6, space="PSUM"))

        wd_all, wu_all, bd_all = [], [], []
        for r in range(RPC):
            wd_sb = wp.tile([128, DC * FK], f32r, tag="wd")
            nc.sync.dma_start(wd_sb[:], wd_d[r])
            wu_sb = wp.tile([128, D], f32r, tag="wu")
            nc.sync.dma_start(wu_sb[:], wu_d[r])
            bd_sb = wp.tile([128, 1], f32, tag="bd")
            nc.sync.dma_start(bd_sb[:], bd_d[r])
            wd_all.append(wd_sb)
            wu_all.append(wu_sb)
            bd_all.append(bd_sb)

        for r in range(RPC):
            wd_sb, wu_sb, bd_sb = wd_all[r], wu_all[r], bd_all[r]
            for n in range(NT_PER_ROW):
                t0 = r * N + n * TT
                # One DMA for the whole 512-token tile in [d, t] layout:
                # xT[(c p), t] -> sbuf [p, (c t)]
                xt_sb = xtp.tile([128, DC * TT], f32r, tag="xt")
                for g in range(2):
                    c0, c1 = g * (DC // 2), (g + 1) * (DC // 2)
                    nc.sync.dma_start(
                        xt_sb[:, c0 * TT: c1 * TT].rearrange(
                            "p (c t) -> p c t", c=DC // 2),
                        xt_d[c0 * 128: c1 * 128, t0: t0 + TT].rearrange(
                            "(c p) t -> p c t", p=128),
                    )

                # Down projection: hT[fk, t] = sum_d wd_cat[d, fk] * xT[d, t]
                h_ps = ps_h.tile([128, TT], f32, tag="hps")
                for c in range(DC):
                    nc.tensor.matmul(
                        h_ps[:],
                        wd_sb[:, c * FK:(c + 1) * FK],
                        xt_sb[:, c * TT:(c + 1) * TT],
                        start=(c == 0),
                        stop=(c == DC - 1),
                    )
                h_sb = hp.tile([128, TT], f32r, tag="h")
                nc.scalar.activation(h_sb[:], h_ps[:], gelu, bias=bd_sb[:, 0:1])

                # Up projection + residual, per 128-wide d chunk (yT layout).
                yt_sb = yp.tile([128, DC * TT], f32, tag="y")
                for c in range(DC):
                    yt_ps = ps_y.tile([128, TT], f32, tag="yps")
                    nc.tensor.matmul(
                        yt_ps[:],
                        wu_sb[:, c * 128:(c + 1) * 128],
                        h_sb[:],
                        start=True, stop=True,
                    )
                    nc.vector.tensor_add(
                        yt_sb[:, c * TT:(c + 1) * TT], yt_ps[:],
                        xt_sb[:, c * TT:(c + 1) * TT].bitcast(f32),
                    )
                for g in range(2):
                    c0, c1 = g * (DC // 2), (g + 1) * (DC // 2)
                    nc.sync.dma_start(
                        yt_d[c0 * 128: c1 * 128, t0: t0 + TT].rearrange(
                            "(c p) t -> p c t", p=128),
                        yt_sb[:, c0 * TT: c1 * TT].rearrange(
                            "p (c t) -> p c t", c=DC // 2),
                    )
    nc.finalize()
    return nc


def _make_runner(nc):
    """Cached jitted SPMD runner (mirrors bass2jax.run_bass_via_pjrt multi-core
    path, but reusable across calls so the NEFF compiles only once)."""
    import jax
    import concourse.mybir as mybir
    from jax.sharding import Mesh, PartitionSpec
    from jax.experimental.shard_map import shard_map
    from concourse import bass2jax

    bass2jax.install_neuronx_cc_hook()

    partition_name = (
        nc.partition_id_tensor.name if nc.partition_id_tensor else None
    )
    in_names: list = []
    out_names: list = []
    out_avals: list = []
    for alloc in nc.m.functions[0].allocations:
        if not isinstance(alloc, mybir.MemoryLocationSet):
            continue
        name = alloc.memorylocations[0].name
        if alloc.kind == "ExternalInput":
            if name != partition_name:
                in_names.append(name)
        elif alloc.kind == "ExternalOutput":
            out_names.append(name)
            out_avals.append(
                jax.core.ShapedArray(
                    tuple(alloc.tensor_shape), mybir.dt.np(alloc.dtype)
                )
            )
    n_params = len(in_names)
    n_outs = len(out_avals)
    all_in_names = tuple(
        in_names + out_names + ([partition_name] if partition_name else [])
    )

    def _body(*args):
        operands = list(args)
        if partition_name is not None:
            operands.append(bass2jax.partition_id_tensor())
        outs = bass2jax._bass_exec_p.bind(
            *operands,
            out_avals=tuple(out_avals),
            in_names=all_in_names,
            out_names=tuple(out_names),
            lowering_input_output_aliases=(),
            sim_require_finite=True,
            sim_require_nnan=True,
            nc=nc,
        )
        return tuple(outs)

    devices = jax.devices()[:NCORES]
    mesh = Mesh(np.asarray(devices), ("core",))
    in_specs = (PartitionSpec("core"),) * (n_params + n_outs)
    out_specs = (PartitionSpec("core"),) * n_outs
    donate = tuple(range(n_params, n_params + n_outs))
    sharded = jax.jit(
        shard_map(_body, mesh=mesh, in_specs=in_specs, out_specs=out_specs,
                  check_rep=False),
        donate_argnums=donate,
        keep_unused=True,
    )

    def run(in_maps):
        concat_in = [
            np.concatenate([np.asarray(m[name]) for m in in_maps], axis=0)
            for name in in_names
        ]
        concat_zeros = [
            np.zeros((NCORES * a.shape[0], *a.shape[1:]), a.dtype)
            for a in out_avals
        ]
        out_arrs = sharded(*concat_in, *concat_zeros)
        return [
            {
                name: np.asarray(out_arrs[i]).reshape(
                    NCORES, *out_avals[i].shape
                )[c]
                for i, name in enumerate(out_names)
            }
            for c in range(NCORES)
        ]

    return run


def _get_state():
    if "run" not in _STATE:
        nc = _build_program()
        _STATE["nc"] = nc
        _STATE["run"] = _make_runner(nc)
    return _STATE


def _gating(tokens, w_gate):
    """Replicates the reference gating math on host (float32 numpy)."""
    pooled = tokens.mean(axis=1, dtype=np.float32)          # [B, D]
    logits = pooled @ w_gate                                # [B, E]
    order = np.argsort(-logits, axis=1, kind="stable")
    topk_idx = order[:, :TOPK]                              # [B, K]
    topk_vals = np.take_along_axis(logits, topk_idx, axis=1)
    mx = topk_vals.max(axis=1, keepdims=True)
    ex = np.exp(topk_vals - mx)
    topk_gates = (ex / ex.sum(axis=1, keepdims=True)).astype(np.float32)
    gates = np.zeros((B, E), np.float32)
    gates[np.arange(B)[:, None], topk_idx] = topk_gates
    importance = gates.sum(axis=0)
    load = (gates > 0).sum(axis=0).astype(np.float32)
    return topk_idx, topk_gates, importance, load


def _pack_inputs(tokens, w_down, b_down, w_up, topk_idx, topk_gates):
    """Builds the 8 per-core input maps (weights gathered per batch row)."""
    sel_wd = w_down[topk_idx]                               # [B, K, D, F]
    wd_cat = np.transpose(sel_wd, (0, 2, 1, 3)).reshape(B, D, FK)
    # lhsT layout: [d_within_chunk, (chunk, fk)]
    wdh = np.ascontiguousarray(
        wd_cat.reshape(B, DC, 128, FK).transpose(0, 2, 1, 3).reshape(B, 128, DC * FK)
    )

    sel_wu = w_up[topk_idx]                                 # [B, K, F, D]
    wu_scaled = sel_wu * topk_gates[:, :, None, None]
    wuh = np.ascontiguousarray(wu_scaled.reshape(B, FK, D))

    bdh = b_down[topk_idx].reshape(B, FK, 1).astype(np.float32)

    in_maps = []
    for c in range(NCORES):
        rows = slice(c * RPC, (c + 1) * RPC)
        in_maps.append({
            "xt": np.ascontiguousarray(tokens[rows].reshape(TOK, D).T),
            "wd": np.ascontiguousarray(wdh[rows]),
            "wu": np.ascontiguousarray(wuh[rows]),
            "bd": np.ascontiguousarray(bdh[rows]),
        })
    return in_maps


def kernel(tokens, w_gate, w_down, b_down, w_up, b_up):
    tokens = np.asarray(tokens, dtype=np.float32)
    w_gate = np.asarray(w_gate, dtype=np.float32)
    w_down = np.asarray(w_down, dtype=np.float32)
    b_down = np.asarray(b_down, dtype=np.float32)
    w_up = np.asarray(w_up, dtype=np.float32)
    b_up = np.asarray(b_up, dtype=np.float32)

    topk_idx, topk_gates, importance, load = _gating(tokens, w_gate)
    in_maps = _pack_inputs(tokens, w_down, b_down, w_up, topk_idx, topk_gates)

    results = _get_state()["run"](in_maps)
    combined = np.concatenate(
        [results[c]["yt"].T for c in range(NCORES)], axis=0
    ).reshape(B, N, D)

    if np.any(b_up):
        brow = (topk_gates[..., None] * b_up[topk_idx]).sum(axis=1)  # [B, D]
        combined = combined + brow[:, None, :]

    return combined, importance, load


# revision 19
# speedup vs baseline: 1.2305x; 1.1227x over previous
"""MoE adapter layer (top-2 routing, bottleneck adapter experts) on 8 trn2 cores.

Strategy: data-parallel over batch rows. Gating (mean-pool -> top-2 -> softmax)
is computed on host (it is tiny); per selected expert the adapter weights are
packed/concatenated on host so each core runs a dense
  y = x + gelu(x @ wd_cat + bd_cat) @ wu_cat_scaled
over its 4 batch rows (4096 tokens). Matmuls run in float32r (full-rate fp32
on the PE, ~1e-4 precision), the residual path stays exact fp32.
importance/load replicate the reference gating math exactly on host.
"""

import numpy as np
import ml_dtypes

# Problem shapes (hardcoded per task contract).
B, N, D, F, E, TOPK = 32, 1024, 768, 64, 8, 2
NCORES = 8
RPC = B // NCORES          # batch rows per core = 4
TOK = RPC * N              # tokens per core = 4096
TT = 512                   # tokens per tile
NT_PER_ROW = N // TT       # 2 tiles per batch row
DC = D // 128              # 6 contraction chunks
FK = TOPK * F              # 128 packed expert features

_STATE: dict = {}


def _build_program():
    import concourse.bacc as bacc
    import concourse.mybir as mybir
    import concourse.tile as tile
    from contextlib import ExitStack

    f32 = mybir.dt.float32
    f32r = mybir.dt.float32r
    bf16 = mybir.dt.bfloat16

    nc = bacc.Bacc("TRN2", target_bir_lowering=False, debug=False)
    # All tensors live in transposed [d, t] layout; host pre/post-transposes.
    xt_d = nc.dram_tensor("xt", [D, TOK], f32r, kind="ExternalInput")
    wd_d = nc.dram_tensor("wd", [RPC, 128, DC * FK], f32r, kind="ExternalInput")
    wu_d = nc.dram_tensor("wu", [RPC, FK, D], f32r, kind="ExternalInput")
    bd_d = nc.dram_tensor("bd", [RPC, FK, 1], f32, kind="ExternalInput")
    yt_d = nc.dram_tensor("yt", [D, TOK], f32, kind="ExternalOutput")

    gelu = mybir.ActivationFunctionType.Gelu_apprx_tanh

    with ExitStack() as ctx:
        tc = ctx.enter_context(tile.TileContext(nc))
        wp = ctx.enter_context(tc.tile_pool(name="w", bufs=4))
        xtp = ctx.enter_context(tc.tile_pool(name="xt", bufs=6))
        hp = ctx.enter_context(tc.tile_pool(name="h", bufs=2))
        yp = ctx.enter_context(tc.tile_pool(name="y", bufs=12))
        ps_h = ctx.enter_context(tc.tile_pool(name="ps_h", bufs=2, space="PSUM"))
        ps_y = ctx.enter_context(tc.tile_pool(name="ps_y", bufs=6, space="PSUM"))

        wd_all, wu_all, bd_all = [], [], []

        def load_weights(r):
            wd_sb = wp.tile([128, DC * FK], f32r, tag="wd")
            nc.sync.dma_start(wd_sb[:], wd_d[r])
            wu_sb = wp.tile([128, D], f32r, tag="wu")
            nc.sync.dma_start(wu_sb[:], wu_d[r])
            bd_sb = wp.tile([128, 1], f32, tag="bd")
            nc.sync.dma_start(bd_sb[:], bd_d[r])
            wd_all.append(wd_sb)
            wu_all.append(wu_sb)
            bd_all.append(bd_sb)

        NTILES = RPC * NT_PER_ROW

        def t0_of(t):
            return (t // NT_PER_ROW) * N + (t % NT_PER_ROW) * TT

        def load_xt(t):
            xt_sb = xtp.tile([128, DC * TT], f32r, tag="xt")
            t0 = t0_of(t)
            for g in range(2):
                c0, c1 = g * (DC // 2), (g + 1) * (DC // 2)
                nc.sync.dma_start(
                    xt_sb[:, c0 * TT: c1 * TT].rearrange(
                        "p (c t) -> p c t", c=DC // 2),
                    xt_d[c0 * 128: c1 * 128, t0: t0 + TT].rearrange(
                        "(c p) t -> p c t", p=128),
                )
            return xt_sb

        # Ramp: row-0 weights + first two token tiles before the rest of the
        # weights, so compute starts as early as possible.
        load_weights(0)
        pending = [load_xt(0), load_xt(1)]
        for r in range(1, RPC):
            load_weights(r)

        for t in range(NTILES):
            xt_sb = pending.pop(0)
            if t + 2 < NTILES:
                pending.append(load_xt(t + 2))
            r = t // NT_PER_ROW
            wd_sb, wu_sb, bd_sb = wd_all[r], wu_all[r], bd_all[r]
            t0 = t0_of(t)

            # Down projection: hT[fk, t] = sum_d wd_cat[d, fk] * xT[d, t]
            h_ps = ps_h.tile([128, TT], f32, tag="hps")
            for c in range(DC):
                nc.tensor.matmul(
                    h_ps[:],
                    wd_sb[:, c * FK:(c + 1) * FK],
                    xt_sb[:, c * TT:(c + 1) * TT],
                    start=(c == 0),
                    stop=(c == DC - 1),
                )
            h_sb = hp.tile([128, TT], f32r, tag="h")
            nc.scalar.activation(h_sb[:], h_ps[:], gelu, bias=bd_sb[:, 0:1])

            # Up projection + residual + store, per 128-wide d chunk
            # (yT layout). Each chunk streams out as soon as its add is done.
            for c in range(DC):
                yt_ps = ps_y.tile([128, TT], f32, tag="yps")
                nc.tensor.matmul(
                    yt_ps[:],
                    wu_sb[:, c * 128:(c + 1) * 128],
                    h_sb[:],
                    start=True, stop=True,
                )
                yt_sb = yp.tile([128, TT], f32, tag="y")
                nc.vector.tensor_add(
                    yt_sb[:], yt_ps[:],
                    xt_sb[:, c * TT:(c + 1) * TT].bitcast(f32),
                )
                nc.sync.dma_start(
                    yt_d[c * 128:(c + 1) * 128, t0: t0 + TT], yt_sb[:]
                )
    nc.finalize()
    return nc


def _make_runner(nc):
    """Cached jitted SPMD runner (mirrors bass2jax.run_bass_via_pjrt multi-core
    path, but reusable across calls so the NEFF compiles only once)."""
    import jax
    import concourse.mybir as mybir
    from jax.sharding import Mesh, PartitionSpec
    from jax.experimental.shard_map import shard_map
    from concourse import bass2jax

    bass2jax.install_neuronx_cc_hook()

    partition_name = (
        nc.partition_id_tensor.name if nc.partition_id_tensor else None
    )
    in_names: list = []
    out_names: list = []
    out_avals: list = []
    for alloc in nc.m.functions[0].allocations:
        if not isinstance(alloc, mybir.MemoryLocationSet):
            continue
        name = alloc.memorylocations[0].name
        if alloc.kind == "ExternalInput":
            if name != partition_name:
                in_names.append(name)
        elif alloc.kind == "ExternalOutput":
            out_names.append(name)
            out_avals.append(
                jax.core.ShapedArray(
                    tuple(alloc.tensor_shape), mybir.dt.np(alloc.dtype)
                )
            )
    n_params = len(in_names)
    n_outs = len(out_avals)
    all_in_names = tuple(
        in_names + out_names + ([partition_name] if partition_name else [])
    )

    def _body(*args):
        operands = list(args)
        if partition_name is not None:
            operands.append(bass2jax.partition_id_tensor())
        outs = bass2jax._bass_exec_p.bind(
            *operands,
            out_avals=tuple(out_avals),
            in_names=all_in_names,
            out_names=tuple(out_names),
            lowering_input_output_aliases=(),
            sim_require_finite=True,
            sim_require_nnan=True,
            nc=nc,
        )
        return tuple(outs)

    devices = jax.devices()[:NCORES]
    mesh = Mesh(np.asarray(devices), ("core",))
    in_specs = (PartitionSpec("core"),) * (n_params + n_outs)
    out_specs = (PartitionSpec("core"),) * n_outs
    donate = tuple(range(n_params, n_params + n_outs))
    sharded = jax.jit(
        shard_map(_body, mesh=mesh, in_specs=in_specs, out_specs=out_specs,
                  check_rep=False),
        donate_argnums=donate,
        keep_unused=True,
    )

    def run(in_maps):
        concat_in = [
            np.concatenate([np.asarray(m[name]) for m in in_maps], axis=0)
            for name in in_names
        ]
        concat_zeros = [
            np.zeros((NCORES * a.shape[0], *a.shape[1:]), a.dtype)
            for a in out_avals
        ]
        out_arrs = sharded(*concat_in, *concat_zeros)
        return [
            {
                name: np.asarray(out_arrs[i]).reshape(
                    NCORES, *out_avals[i].shape
                )[c]
                for i, name in enumerate(out_names)
            }
            for c in range(NCORES)
        ]

    return run


def _get_state():
    if "run" not in _STATE:
        nc = _build_program()
        _STATE["nc"] = nc
        _STATE["run"] = _make_runner(nc)
    return _STATE


def _gating(tokens, w_gate):
    """Replicates the reference gating math on host (float32 numpy)."""
    pooled = tokens.mean(axis=1, dtype=np.float32)          # [B, D]
    logits = pooled @ w_gate                                # [B, E]
    order = np.argsort(-logits, axis=1, kind="stable")
    topk_idx = order[:, :TOPK]                              # [B, K]
    topk_vals = np.take_along_axis(logits, topk_idx, axis=1)
    mx = topk_vals.max(axis=1, keepdims=True)
    ex = np.exp(topk_vals - mx)
    topk_gates = (ex / ex.sum(axis=1, keepdims=True)).astype(np.float32)
    gates = np.zeros((B, E), np.float32)
    gates[np.arange(B)[:, None], topk_idx] = topk_gates
    importance = gates.sum(axis=0)
    load = (gates > 0).sum(axis=0).astype(np.float32)
    return topk_idx, topk_gates, importance, load


def _pack_inputs(tokens, w_down, b_down, w_up, topk_idx, topk_gates):
    """Builds the 8 per-core input maps (weights gathered per batch row)."""
    sel_wd = w_down[topk_idx]                               # [B, K, D, F]
    wd_cat = np.transpose(sel_wd, (0, 2, 1, 3)).reshape(B, D, FK)
    # lhsT layout: [d_within_chunk, (chunk, fk)]
    wdh = np.ascontiguousarray(
        wd_cat.reshape(B, DC, 128, FK).transpose(0, 2, 1, 3).reshape(B, 128, DC * FK)
    )

    sel_wu = w_up[topk_idx]                                 # [B, K, F, D]
    wu_scaled = sel_wu * topk_gates[:, :, None, None]
    wuh = np.ascontiguousarray(wu_scaled.reshape(B, FK, D))

    bdh = b_down[topk_idx].reshape(B, FK, 1).astype(np.float32)

    in_maps = []
    for c in range(NCORES):
        rows = slice(c * RPC, (c + 1) * RPC)
        in_maps.append({
            "xt": np.ascontiguousarray(tokens[rows].reshape(TOK, D).T),
            "wd": np.ascontiguousarray(wdh[rows]),
            "wu": np.ascontiguousarray(wuh[rows]),
            "bd": np.ascontiguousarray(bdh[rows]),
        })
    return in_maps


def kernel(tokens, w_gate, w_down, b_down, w_up, b_up):
    tokens = np.asarray(tokens, dtype=np.float32)
    w_gate = np.asarray(w_gate, dtype=np.float32)
    w_down = np.asarray(w_down, dtype=np.float32)
    b_down = np.asarray(b_down, dtype=np.float32)
    w_up = np.asarray(w_up, dtype=np.float32)
    b_up = np.asarray(b_up, dtype=np.float32)

    topk_idx, topk_gates, importance, load = _gating(tokens, w_gate)
    in_maps = _pack_inputs(tokens, w_down, b_down, w_up, topk_idx, topk_gates)

    results = _get_state()["run"](in_maps)
    combined = np.concatenate(
        [results[c]["yt"].T for c in range(NCORES)], axis=0
    ).reshape(B, N, D)

    if np.any(b_up):
        brow = (topk_gates[..., None] * b_up[topk_idx]).sum(axis=1)  # [B, D]
        combined = combined + brow[:, None, :]

    return combined, importance, load
